# revision 53
# baseline (speedup 1.0000x reference)
"""Trainium2 Bass kernel for nn_MultiHeadedAttentionWithGate (v2).

Math (per molecule, validated against reference): the reference's
reshapes are flat views, so with u = "virtual row" (1024 per molecule),
the computation is per-u over contiguous flat segments: K/V/M rows of
320 (10 nei x 32), X rows of 640 (10 x 64), q rows of 32.

Phase decomposition: u = 4*g + r.  For fixed phase r (0..3) and g on
partitions, every tensor's u-row is a contiguous DRAM segment and the
K/V/M[u-layout] projections decompose into matmuls over X^T chunks
(the f16 PE-transposes of the per-phase Xu tiles chunked by 128 cols).

v2 schedule (vs v1):
  - V psum columns packed (dk, nei)-major so the attention-weighted
    reduce is contiguous; a ones-column folds the softmax denominator
    into the same reduce.
  - One fused ACT copy drains V+K psum -> SBUF f16; score mul runs on
    DVE in f16 2x mode; amul on GpSimd; the three segment reduces stay
    on DVE (the only engine with free-axis reduce).
  - gate "cur" dot is folded into the q-projection matmul (8 extra
    columns) instead of a per-G vector pass.
  - all DMA descriptor issues moved to the Sync engine.
  - next-G-block PE transposes are software-pipelined one per phase.
  - per-molecule q-prepass is interleaved with the previous molecule.

Sharding: data-parallel over batch: 8 molecules per core x 8 cores.
"""

import sys

for _p in ("/opt/trn_rl_repo", "/root/.axon_site/_ro/trn_rl_repo"):
    if _p not in sys.path:
        sys.path.insert(0, _p)

from contextlib import ExitStack

import numpy as np

import concourse.bass as bass
import concourse.mybir as mybir
from concourse import bacc
from concourse.tile import TileContext

F16 = mybir.dt.float16
F32 = mybir.dt.float32
EXP = mybir.ActivationFunctionType.Exp
ADD = mybir.AluOpType.add
MAX = mybir.AluOpType.max
AXL_X = mybir.AxisListType.X

N_CORES = 8
BM = 8          # molecules per core
A = 128         # atoms
NEI = 10
D = 256
D2 = 512
NBLK = 2 * BM   # g-blocks per core (2 per molecule)


def _wa(r):
    return 256 - 64 * r


def _seg_ranges(r):
    """s-intervals of the 320-wide segment and their X^T d-chunk."""
    wa = _wa(r)
    return [(r, 0, wa), (r + 1, wa, 320)]


def _e_of(r, s):
    """weight-matrix column for segment position s of phase r."""
    wa = _wa(r)
    return 64 * r + s if s < wa else s - wa


DEBUG = False


def build_nc(with_bias: bool, bg_val: float) -> bass.Bass:
    nc = bacc.Bacc("TRN2", target_bir_lowering=False)

    dbg = {}
    if DEBUG:
        for nm, shp, dt in [
                ("dbg_qu16", [128, 4, 32], F16), ("dbg_qg", [128, 4], F32),
                ("dbg_kv16", [128, 2, 330], F16),
                ("dbg_smul", [128, 10, 32], F16),
                ("dbg_score", [128, 10], F32), ("dbg_ex", [128, 10], F16),
                ("dbg_amul", [128, 33, 10], F16),
                ("dbg_araw", [128, 4, 33], F32),
                ("dbg_emax", [128, 4, 32], F32),
                ("dbg_gave", [128, 4], F32), ("dbg_eg", [128, 4], F32),
                ("dbg_kvm", [128, 3, 320], F32),
                ("dbg_xt", [128, 128], F16)]:
            dbg[nm] = nc.declare_dram_parameter(nm, shp, dt, isOutput=True)

    x_h = nc.declare_dram_parameter("x", [BM, A * NEI, D2], F32, isOutput=False)
    qin_h = nc.declare_dram_parameter("qin", [BM, A, D], F32, isOutput=False)
    ident_h = nc.declare_dram_parameter("ident", [128, 128], F16, isOutput=False)
    ident32_h = nc.declare_dram_parameter("ident32", [128, 128], F32,
                                          isOutput=False)
    wq_h = nc.declare_dram_parameter("wq", [128, 2, 264], F16, isOutput=False)
    wk_h = nc.declare_dram_parameter("wk", [128, 4, 256], F16, isOutput=False)
    wv_h = nc.declare_dram_parameter("wv", [128, 4, 4, 320], F16, isOutput=False)
    wm_h = nc.declare_dram_parameter("wm", [128, 4, 256], F16, isOutput=False)
    wgav_h = nc.declare_dram_parameter("wgav", [128, 1], F16, isOutput=False)
    if bg_val != 0.0:
        bgc_h = nc.declare_dram_parameter("bgc", [128, 1], F32, isOutput=False)
    wge_h = nc.declare_dram_parameter("wge", [128, 32], F32, isOutput=False)
    ssel_h = nc.declare_dram_parameter("ssel", [128, 32], F32, isOutput=False)
    s2sel_h = nc.declare_dram_parameter("s2sel", [32, 128], F32, isOutput=False)
    if with_bias:
        bkvm_h = nc.declare_dram_parameter("bkvm", [1, 3, 256], F16,
                                           isOutput=False)
        bvp_h = nc.declare_dram_parameter("bvp", [1, 4, 320], F16,
                                          isOutput=False)
        bq_h = nc.declare_dram_parameter("bq", [1, 264], F16, isOutput=False)
        ones_h = nc.declare_dram_parameter("ones", [1, 128], F16,
                                           isOutput=False)
    out_h = nc.declare_dram_parameter("out", [BM, A, D], F32, isOutput=True)

    # flat per-molecule views
    xg = (x_h[:].rearrange("b n c -> b (n c)")
          .rearrange("b (g p r t) -> b g p r t", g=2, p=128, r=4, t=640))
    o5 = (out_h[:].rearrange("b a c -> b (a c)")
          .rearrange("b (g p r k) -> b g p r k", g=2, p=128, r=4, k=32))

    with TileContext(nc) as tc, ExitStack() as ctx:
        consts = ctx.enter_context(tc.tile_pool(name="consts", bufs=1))
        sb_x = ctx.enter_context(tc.tile_pool(name="sbx", bufs=4))
        sb_xt = ctx.enter_context(tc.tile_pool(name="sbxt", bufs=9))
        sb_kv = ctx.enter_context(tc.tile_pool(name="sbkv", bufs=3))
        sb_ew = ctx.enter_context(tc.tile_pool(name="sbew", bufs=3))
        sb_g = ctx.enter_context(tc.tile_pool(name="sbg", bufs=3))
        sb_q = ctx.enter_context(tc.tile_pool(name="sbq", bufs=2))
        ps_p = ctx.enter_context(tc.tile_pool(name="pp", bufs=2, space="PSUM"))
        ps_t = ctx.enter_context(tc.tile_pool(name="pt", bufs=1, space="PSUM"))
        ps_m = ctx.enter_context(tc.tile_pool(name="pm", bufs=1, space="PSUM"))
        dram = ctx.enter_context(tc.tile_pool(name="dram", bufs=1,
                                              space="DRAM"))

        def cload(h, shape, dtype):
            t = consts.tile(shape, dtype, tag=h.name, name=h.name)
            nc.sync.dma_start(out=t, in_=h[:])
            return t

        # const load order matters at startup: transposes need ident, the
        # first phase's matmuls need wgav/wk/wm and wv's r=0 slice.
        ident_t = cload(ident_h, [128, 128], F16)
        ident32_t = cload(ident32_h, [128, 128], F32)
        wgav_t = cload(wgav_h, [128, 1], F16)
        wq_t = cload(wq_h, [128, 2, 264], F16)
        wk_t = cload(wk_h, [128, 4, 256], F16)
        wm_t = cload(wm_h, [128, 4, 256], F16)
        wv_t = consts.tile([128, 4, 4, 320], F16, tag="wv", name="wv")
        nc.sync.dma_start(out=wv_t[:, 0], in_=wv_h[:, 0])
        bgc_t = cload(bgc_h, [128, 1], F32) if bg_val != 0.0 else None
        wge_t = cload(wge_h, [128, 32], F32)
        ssel_t = cload(ssel_h, [128, 32], F32)
        s2sel_t = cload(s2sel_h, [32, 128], F32)
        nc.sync.dma_start(out=wv_t[:, 1:4], in_=wv_h[:, 1:4])
        if with_bias:
            bkvm_t = cload(bkvm_h, [1, 3, 256], F16)
            bvp_t = cload(bvp_h, [1, 4, 320], F16)
            bq_t = cload(bq_h, [1, 264], F16)
            ones_t = cload(ones_h, [1, 128], F16)

        qdram = dram.tile([BM, 128, 256], F16)
        qdramg = dram.tile([BM, 128, 8], F32)

        # kv16 slot layout: [, 0, 0:10] ones (softmax denominator rides the
        # attention reduce), [, 0, 10:330] V, [, 0, 330] pg column,
        # [, 1, 10:330] K.  Ones are memset once per slot.
        KV_BUFS = 3
        for _ in range(KV_BUFS):
            kvi = sb_kv.tile([128, 2, 336], F16, tag="kv16", name="kv16",
                             bufs=KV_BUFS)
            nc.vector.memset(kvi[:, 0, 0:10], 1.0)

        def prepass(mol):
            """q/gate-cur projection for one molecule -> qdram[mol]."""
            qin32 = sb_q.tile([128, 256], F32, tag="qin32", name="qin32")
            nc.sync.dma_start(out=qin32, in_=qin_h[mol])
            qtp = ps_t.tile([128, 2, 128], F32, tag="pt", name="qtp")
            for w in range(2):
                nc.tensor.transpose(qtp[:, w, :],
                                    qin32[:, 128 * w:128 * (w + 1)], ident32_t)
            qT = sb_q.tile([128, 2, 128], F16, tag="qT", name="qT")
            nc.vector.tensor_copy(out=qT, in_=qtp)
            qpsum = ps_m.tile([128, 264], F32, tag="pm", name="qpsum")
            nc.tensor.matmul(qpsum, qT[:, 0, :], wq_t[:, 0, :],
                             start=True, stop=False)
            nc.tensor.matmul(qpsum, qT[:, 1, :], wq_t[:, 1, :],
                             start=False, stop=not with_bias)
            if with_bias:
                nc.tensor.matmul(qpsum, ones_t, bq_t, start=False, stop=True)
            qnat = sb_q.tile([128, 256], F16, tag="qnat", name="qnat")
            nc.scalar.copy(out=qnat, in_=qpsum[:, 0:256])
            nc.sync.dma_start(out=qdram[mol], in_=qnat)
            qng = sb_q.tile([128, 8], F32, tag="qng", name="qng")
            nc.scalar.copy(out=qng, in_=qpsum[:, 256:264])
            nc.sync.dma_start(out=qdramg[mol], in_=qng)

        # ---------------- per-block state ----------------
        xgt = {}      # block -> x16 tile [128, 4, 640]
        xts = {}      # block -> list of 4 XT sbuf tiles [128, 5, 128]
        qu16s = {}    # block -> [128, 4, 32] f16
        qgs = {}      # block -> [128, 4] f32
        gaveGs = {}   # block -> [128, 4] f32 (neighbor-mean gate dot)
        kvms = {}     # phase -> kvm psum tile
        kv16s = {}    # phase -> kv16 sbuf tile
        amuls = {}    # phase -> amul16 tile
        arawGs = {}   # block -> [128, 4, 33] f32 (col 0 = denominator)
        emaxGs = {}   # block -> [128, 4, 32] f32
        gtail = {}    # block -> dict of gate tiles

        def issue_x(bg, parts=1):
            """parts>1 splits the load by row-phase so the first transposes
            can start before the whole block has landed (startup only)."""
            mol, G = divmod(bg, 2)
            t = sb_x.tile([128, 4, 640], F16, tag="x16", name="x16")
            step = 4 // parts
            for r0 in range(0, 4, step):
                nc.gpsimd.dma_start(out=t[:, r0:r0 + step, :],
                                    in_=xg[mol, G, :, r0:r0 + step, :])
            xgt[bg] = t

        def issue_q(bg):
            # u-layout gather: partition p reads qdram row 64G + p//2,
            # column block 128*(p%2) -- two DMAs, one per parity.
            mol, G = divmod(bg, 2)
            qt = sb_ew.tile([128, 4, 32], F16, tag="qu16", name="qu16")
            gt = sb_g.tile([128, 4], F32, tag="qg", name="qg")
            qts = qt.rearrange("(p2 pb) r k -> pb p2 r k", pb=2)
            gts = gt.rearrange("(p2 pb) r -> pb p2 r", pb=2)
            for pb in range(2):
                src = (qdram[mol, 64 * G:64 * G + 64,
                             128 * pb:128 * pb + 128]
                       .rearrange("p (r k) -> p r k", r=4))
                nc.sync.dma_start(out=qts[pb], in_=src)
                nc.sync.dma_start(
                    out=gts[pb],
                    in_=qdramg[mol, 64 * G:64 * G + 64,
                               4 * pb:4 * pb + 4])
            qu16s[bg] = qt
            qgs[bg] = gt

        def t_batch(bg, r):
            """PE transposes of x block bg, Xu row-phase r + ACT copy."""
            tp = ps_t.tile([128, 5, 128], F16, tag="pt", name="tp")
            for w in range(5):
                nc.tensor.transpose(tp[:, w, :],
                                    xgt[bg][:, r, 128 * w:128 * (w + 1)],
                                    ident_t)
            xtb = sb_xt.tile([128, 5, 128], F16, tag="xt", name="xtb")
            nc.scalar.copy(out=xtb, in_=tp)
            if bg not in xts:
                xts[bg] = [None] * 4
            xts[bg][r] = xtb

        def XT(bg, d, fc):
            cid = 4 * d + fc
            return xts[bg][cid // 5][:, cid % 5, :]

        def phase_matmuls(i):
            bg, r = divmod(i, 4)
            kvm = ps_p.tile([128, 3, 512], F32, tag="pp", name="kvm")
            kvms[i] = kvm
            wa = _wa(r)
            nA = wa // 32
            # One accumulation group (start..stop) at a time per psum bank:
            # ranges outer, contraction chunks (fc) inner.
            # V: bank 0, (dk, nei)-packed columns.  The per-phase weight
            # pack stores each range's columns contiguously in (k, n)
            # enumeration order so the moving-operand fetch is sequential.
            vout = kvm[:, 0, 0:320].rearrange("p (k n) -> p k n", k=32)
            for (d, n0, n1) in ((r, 0, nA), (r + 1, nA, 10)):
                nr = n1 - n0
                wslc = (wv_t[:, r, :, 32 * n0:32 * n0 + 32 * nr]
                        .rearrange("p f (k n) -> p f k n", n=nr))
                for fc in range(4):
                    st, sp = fc == 0, (fc == 3) and not with_bias
                    nc.tensor.matmul(vout[:, :, n0:n1], XT(bg, d, fc),
                                     wslc[:, fc], start=st, stop=sp)
                if with_bias:
                    nc.tensor.matmul(
                        vout[:, :, n0:n1], ones_t,
                        bvp_t[:, r, 32 * n0:32 * n0 + 32 * nr]
                        .rearrange("o (k n) -> o k n", n=nr),
                        start=False, stop=True)
            # K: bank 1, segment order; weight col e0 = 64r (d=r) / 0 (d=r+1)
            wa = _wa(r)
            for (d, s0, s1) in _seg_ranges(r):
                e0 = 64 * r - s0 if d == r else -wa
                for fc in range(4):
                    st, sp = fc == 0, (fc == 3) and not with_bias
                    nc.tensor.matmul(kvm[:, 1, s0:s1], XT(bg, d, fc),
                                     wk_t[:, fc, s0 + e0:s1 + e0],
                                     start=st, stop=sp)
                if with_bias:
                    nc.tensor.matmul(kvm[:, 1, s0:s1], ones_t,
                                     bkvm_t[:, 1, s0 + e0:s1 + e0],
                                     start=False, stop=True)
            # M: bank 2, segment order
            for (d, s0, s1) in _seg_ranges(r):
                e0 = 64 * r - s0 if d == r else -wa
                for fc in range(4):
                    st, sp = fc == 0, (fc == 3) and not with_bias
                    nc.tensor.matmul(kvm[:, 2, s0:s1], XT(bg, d, fc),
                                     wm_t[:, fc, s0 + e0:s1 + e0],
                                     start=st, stop=sp)
                if with_bias:
                    nc.tensor.matmul(kvm[:, 2, s0:s1], ones_t,
                                     bkvm_t[:, 2, s0 + e0:s1 + e0],
                                     start=False, stop=True)
            # neighbor-mean gate dot, folded into the PE pass; accumulates
            # in the V bank's spare column (extracted with the kv copy)
            for j, cid in enumerate(range(5 * r, 5 * r + 5)):
                d, fc = divmod(cid, 4)
                nc.tensor.matmul(kvm[:, 0, 320:321], XT(bg, d, fc), wgav_t,
                                 start=(j == 0), stop=(j == 4),
                                 skip_group_check=True)

        def phase_front(i):
            """emax + kv drain + score chain for phase i (DVE/ACT)."""
            bg, r = divmod(i, 4)
            kvm = kvms[i]
            if bg not in emaxGs:
                emaxGs[bg] = sb_ew.tile([128, 4, 32], F32, tag="emaxG",
                                        name="emaxG", bufs=2)
                arawGs[bg] = sb_ew.tile([128, 4, 33], F32, tag="arawG",
                                        name="arawG", bufs=3)
            nc.vector.tensor_reduce(
                out=emaxGs[bg][:, r, :],
                in_=kvm[:, 2, 0:320].rearrange("p (j k) -> p k j", j=10),
                axis=AXL_X, op=MAX)

        def phase_score(i):
            bg, r = divmod(i, 4)
            kvm = kvms.pop(i)
            kv16 = sb_kv.tile([128, 2, 336], F16, tag="kv16", name="kv16",
                              bufs=KV_BUFS)
            kv16s[i] = kv16
            if DEBUG and i == 0:
                kvmc = sb_ew.tile([128, 3, 320], F32, tag="dbgkvm",
                                  name="kvmc", bufs=1)
                nc.vector.tensor_copy(out=kvmc, in_=kvm[:, :, 0:320])
                nc.sync.dma_start(out=dbg["dbg_kvm"][:], in_=kvmc)
            nc.scalar.copy(out=kv16[:, :, 10:331], in_=kvm[:, 0:2, 0:321])
            if bg not in gaveGs:
                gaveGs[bg] = sb_g.tile([128, 4], F32, tag="gaveG",
                                       name="gaveG", bufs=2)
            nc.gpsimd.tensor_copy(out=gaveGs[bg][:, r:r + 1],
                                  in_=kv16[:, 0, 330:331])
            smul = sb_ew.tile([128, 10, 32], F16, tag="smul", name="smul")
            nc.vector.tensor_mul(
                smul, kv16[:, 1, 10:330].rearrange("p (j k) -> p j k", j=10),
                qu16s[bg][:, r, :].unsqueeze(1).broadcast_to([128, 10, 32]))
            score = sb_ew.tile([128, 10], F32, tag="score", name="score")
            nc.vector.tensor_reduce(out=score, in_=smul, axis=AXL_X, op=ADD)
            ex = sb_ew.tile([128, 10], F16, tag="ex", name="ex")
            nc.scalar.activation(out=ex, in_=score, func=EXP)
            amul = sb_ew.tile([128, 33, 10], F16, tag="amul", name="amul")
            nc.gpsimd.tensor_mul(
                amul, kv16[:, 0, 0:330].rearrange("p (k n) -> p k n", k=33),
                ex.unsqueeze(1).broadcast_to([128, 33, 10]))
            amuls[i] = amul
            if DEBUG and i == 0:
                nc.sync.dma_start(out=dbg["dbg_kv16"][:], in_=kv16[:, :, 0:330])
                nc.sync.dma_start(out=dbg["dbg_smul"][:], in_=smul)
                nc.sync.dma_start(out=dbg["dbg_score"][:], in_=score)
                nc.sync.dma_start(out=dbg["dbg_ex"][:], in_=ex)
                nc.sync.dma_start(out=dbg["dbg_amul"][:], in_=amul)
                nc.sync.dma_start(out=dbg["dbg_qu16"][:], in_=qu16s[bg])
                nc.sync.dma_start(out=dbg["dbg_qg"][:], in_=qgs[bg])
                nc.sync.dma_start(out=dbg["dbg_xt"][:], in_=XT(bg, 0, 0))

        def phase_araw(i):
            bg, r = divmod(i, 4)
            nc.vector.tensor_reduce(out=arawGs[bg][:, r, :],
                                    in_=amuls.pop(i), axis=AXL_X, op=ADD)
            kv16s.pop(i)

        def g_tail_eg(bg):
            """gate logits for a finished g-block (independent of araw)."""
            emaxp = sb_g.tile([128, 4, 32], F32, tag="emaxp", name="emaxp",
                              bufs=2)
            nc.gpsimd.tensor_mul(
                emaxp, emaxGs[bg],
                wge_t.unsqueeze(1).broadcast_to([128, 4, 32]))
            gemxB = sb_g.tile([128, 4], F32, tag="gemxB", name="gemxB",
                              bufs=2)
            nc.vector.tensor_reduce(out=gemxB, in_=emaxp, axis=AXL_X, op=ADD)
            gl1 = sb_g.tile([128, 4], F32, tag="gl1", name="gl1", bufs=2)
            nc.gpsimd.tensor_add(gl1, qgs.pop(bg), gemxB)
            gl2 = sb_g.tile([128, 4], F32, tag="gl2", name="gl2", bufs=2)
            nc.gpsimd.tensor_add(gl2, gl1, gaveGs[bg])
            egB = sb_g.tile([128, 4], F32, tag="egB", name="egB", bufs=3)
            if bg_val != 0.0:
                nc.scalar.activation(out=egB, in_=gl2, func=EXP, bias=bgc_t)
            else:
                nc.scalar.activation(out=egB, in_=gl2, func=EXP)
            gtail[bg] = {"egB": egB}
            if DEBUG and bg == 0:
                nc.sync.dma_start(out=dbg["dbg_emax"][:], in_=emaxGs[bg])
                nc.sync.dma_start(out=dbg["dbg_gave"][:], in_=gaveGs[bg])
                nc.sync.dma_start(out=dbg["dbg_eg"][:], in_=egB)

        def g_tail_ra(bg):
            raB = sb_g.tile([128, 4], F32, tag="raB", name="raB", bufs=3)
            nc.vector.reciprocal(out=raB, in_=arawGs[bg][:, :, 0])
            gtail[bg]["raB"] = raB
            if DEBUG and bg == 0:
                nc.sync.dma_start(out=dbg["dbg_araw"][:], in_=arawGs[bg])

        def mol_tail_a(mol):
            """head-sum of gate numerators (PE) + reciprocal."""
            g0, g1 = gtail[2 * mol], gtail[2 * mol + 1]
            gd = ps_m.tile([32, 4], F32, tag="pm", name="gd")
            for r in range(4):
                nc.tensor.matmul(gd[:, r:r + 1], ssel_t,
                                 g0["egB"][:, r:r + 1],
                                 start=True, stop=False)
                nc.tensor.matmul(gd[:, r:r + 1], ssel_t,
                                 g1["egB"][:, r:r + 1],
                                 start=False, stop=True)
            rg = sb_g.tile([32, 4], F32, tag="rg", name="rg", bufs=2)
            nc.vector.reciprocal(out=rg, in_=gd)
            return rg

        def mol_tail_b(mol, rg):
            """broadcast the head-sum back and scale the raw attention."""
            g0, g1 = gtail.pop(2 * mol), gtail.pop(2 * mol + 1)
            inv = ps_m.tile([128, 4], F32, tag="pm", name="inv")
            for r in range(4):
                nc.tensor.matmul(inv[:, r:r + 1], s2sel_t, rg[:, r:r + 1],
                                 start=True, stop=True)
            invs = sb_g.tile([128, 4], F32, tag="invs", name="invs", bufs=2)
            nc.scalar.copy(out=invs, in_=inv)
            for gg, gt in ((0, g0), (1, g1)):
                bg = 2 * mol + gg
                t1 = sb_g.tile([128, 4], F32, tag="t1", name="t1", bufs=2)
                nc.gpsimd.tensor_mul(t1, invs, gt["raB"])
                c2 = sb_g.tile([128, 4], F32, tag="c2", name="c2", bufs=2)
                nc.gpsimd.tensor_mul(c2, t1, gt["egB"])
                outB = sb_g.tile([128, 4, 32], F32, tag="outB", name="outB",
                                 bufs=4)
                nc.gpsimd.tensor_mul(
                    outB, arawGs.pop(bg)[:, :, 1:33],
                    c2.unsqueeze(2).broadcast_to([128, 4, 32]))
                nc.sync.dma_start(out=o5[mol, gg], in_=outB)
                emaxGs.pop(bg, None)
                gaveGs.pop(bg, None)

        # ---------------- prologue ----------------
        # x loads own the (gpsimd-issued) cast ring entirely; q traffic is
        # all on the fast hardware ring.  The first two blocks' loads are
        # split per row-phase and interleaved so transposes start early.
        xgt[0] = sb_x.tile([128, 4, 640], F16, tag="x16", name="x16")
        xgt[1] = sb_x.tile([128, 4, 640], F16, tag="x16", name="x16")
        for rr, bb in ((0, 0), (1, 0), (0, 1), (2, 0), (1, 1), (3, 0),
                       (2, 1), (3, 1)):
            mol_, G_ = divmod(bb, 2)
            nc.gpsimd.dma_start(out=xgt[bb][:, rr:rr + 1, :],
                                in_=xg[mol_, G_, :, rr:rr + 1, :])
        for m in range(BM):
            prepass(m)
        issue_q(0)
        for r in range(4):
            t_batch(0, r)

        NPH = 4 * NBLK
        pend_b = {}                     # stage idx -> (mol, rg16)
        for i in range(NPH):
            bg, r = divmod(i, 4)
            mol, G = divmod(bg, 2)
            if r == 0 and bg + 2 < NBLK:
                issue_x(bg + 2)
            # PE: transposes for phase i+4's block, one batch per phase
            if i + 4 < NPH:
                bg2, r2 = divmod(i + 4, 4)
                t_batch(bg2, r2)
            if i > 0 and r == 0:
                g_tail_eg(bg - 1)      # independent of araw; feeds gd early
            if i > 0:
                phase_araw(i - 1)
                if r == 0:
                    g_tail_ra(bg - 1)
            phase_matmuls(i)
            if i > 0 and r == 0 and G == 0 and mol > 0:
                pend_b[i + 1] = (mol - 1, mol_tail_a(mol - 1))
            if i in pend_b:             # PE inv one stage after gd
                m_, rg16_ = pend_b.pop(i)
                mol_tail_b(m_, rg16_)
            phase_front(i)
            phase_score(i)
            if r == 1 and bg + 1 < NBLK:
                issue_q(bg + 1)
        phase_araw(NPH - 1)
        g_tail_eg(NBLK - 1)
        g_tail_ra(NBLK - 1)
        mol_tail_b(BM - 1, mol_tail_a(BM - 1))

    nc.finalize()
    return nc


def _prep_consts(Wq, bq, Wk, bk, Wv, bv, Wam, bam, Wg, bg):
    Wq = np.asarray(Wq, np.float32)
    Wk = np.asarray(Wk, np.float32)
    Wv = np.asarray(Wv, np.float32)
    Wam = np.asarray(Wam, np.float32)
    wg = np.asarray(Wg, np.float32).reshape(-1)

    # q projection + gate-cur block-diagonal columns
    wq = np.zeros((128, 2, 264), np.float32)
    for c in range(2):
        wq[:, c, 0:256] = Wq[128 * c:128 * (c + 1), :]
        for p in range(128):
            ch = 128 * c + p
            wq[p, c, 256 + ch // 32] = wg[ch % 32]

    # K/M natural packs (row-chunked): pack[p, fc, c] = W[128*fc + p, c]
    wk_c = np.empty((128, 4, 256), np.float32)
    wm_c = np.empty((128, 4, 256), np.float32)
    for fc in range(4):
        rows = slice(128 * fc, 128 * (fc + 1))
        wk_c[:, fc, :] = Wk[rows, :]
        wm_c[:, fc, :] = Wam[rows, :]
    # V per-phase pack, each range's columns contiguous in (k, n) order
    wv_c = np.empty((128, 4, 4, 320), np.float32)
    for r in range(4):
        wa = 256 - 64 * r
        nA = wa // 32
        cols = np.empty((32, NEI), np.int64)
        for k in range(32):
            for n in range(NEI):
                sv = 32 * n + k
                cols[k, n] = 64 * r + sv if sv < wa else sv - wa
        order = np.concatenate([cols[:, 0:nA].reshape(-1),
                                cols[:, nA:NEI].reshape(-1)])
        for fc in range(4):
            rows = slice(128 * fc, 128 * (fc + 1))
            wv_c[:, r, fc, :] = Wv[rows, :][:, order]

    p = np.arange(128)
    ssel = (p[:, None] % 32 == np.arange(32)[None, :]).astype(np.float32)
    consts = {
        "ident": np.eye(128, dtype=np.float16),
        "ident32": np.eye(128, dtype=np.float32),
        "wq": wq.astype(np.float16),
        "wk": wk_c.astype(np.float16),
        "wv": wv_c.astype(np.float16),
        "wm": wm_c.astype(np.float16),
        "wgav": (wg[64 + (p % 64)] / NEI).astype(np.float16).reshape(128, 1),
        "wge": np.tile(wg[32:64], (128, 1)).astype(np.float32),
        "ssel": ssel,
        "s2sel": ssel.T.copy(),
    }
    bg_val = float(np.asarray(bg).reshape(-1)[0])
    if bg_val != 0.0:
        consts["bgc"] = np.full((128, 1), bg_val, np.float32)
    with_bias = any(np.any(np.asarray(b) != 0) for b in (bq, bk, bv, bam))
    if with_bias:
        bkvm = np.stack([np.asarray(bv, np.float32),
                         np.asarray(bk, np.float32),
                         np.asarray(bam, np.float32)])[None]
        consts["bkvm"] = bkvm.astype(np.float16)
        bvp = np.empty((1, 4, 320), np.float32)
        bvf = np.asarray(bv, np.float32)
        for r in range(4):
            wa = 256 - 64 * r
            nA = wa // 32
            cols = np.empty((32, NEI), np.int64)
            for k in range(32):
                for n in range(NEI):
                    sv = 32 * n + k
                    cols[k, n] = 64 * r + sv if sv < wa else sv - wa
            order = np.concatenate([cols[:, 0:nA].reshape(-1),
                                    cols[:, nA:NEI].reshape(-1)])
            bvp[0, r, :] = bvf[order]
        consts["bvp"] = bvp.astype(np.float16)
        bqp = np.zeros((1, 264), np.float32)
        bqp[0, 0:256] = np.asarray(bq, np.float32)
        consts["bq"] = bqp.astype(np.float16)
        consts["ones"] = np.ones((1, 128), np.float16)
    return consts, with_bias, float(np.asarray(bg).reshape(-1)[0])


_CACHE = {}
TRACE = False       # set by test.py for profiling runs
LAST_RESULTS = None  # BassKernelResults from the most recent run


def kernel(input_multihead, input_q, Wq, bq, Wk, bk, Wv, bv, Wam, bam, Wg, bg):
    from concourse.bass_utils import run_bass_kernel_spmd

    consts, with_bias, bg_val = _prep_consts(
        Wq, bq, Wk, bk, Wv, bv, Wam, bam, Wg, bg)

    key = (with_bias, bg_val, DEBUG)
    if key not in _CACHE:
        _CACHE[key] = build_nc(with_bias, bg_val)
    nc = _CACHE[key]

    x = np.ascontiguousarray(np.asarray(input_multihead, np.float32))
    q = np.ascontiguousarray(np.asarray(input_q, np.float32))
    in_maps = []
    for c in range(N_CORES):
        m = {"x": x[BM * c:BM * (c + 1)], "qin": q[BM * c:BM * (c + 1)]}
        m.update(consts)
        in_maps.append(m)

    res = run_bass_kernel_spmd(nc, in_maps, list(range(N_CORES)), trace=TRACE)
    global LAST_RESULTS
    LAST_RESULTS = res
    return np.concatenate([res.results[c]["out"] for c in range(N_CORES)],
                          axis=0)


# revision 65
# speedup vs baseline: 1.1039x; 1.1039x over previous
"""Trainium2 Bass kernel for nn_MultiHeadedAttentionWithGate (v2).

Math (per molecule, validated against reference): the reference's
reshapes are flat views, so with u = "virtual row" (1024 per molecule),
the computation is per-u over contiguous flat segments: K/V/M rows of
320 (10 nei x 32), X rows of 640 (10 x 64), q rows of 32.

Phase decomposition: u = 4*g + r.  For fixed phase r (0..3) and g on
partitions, every tensor's u-row is a contiguous DRAM segment and the
K/V/M[u-layout] projections decompose into matmuls over X^T chunks
(the f16 PE-transposes of the per-phase Xu tiles chunked by 128 cols).

v2 schedule (vs v1):
  - V psum columns packed (dk, nei)-major so the attention-weighted
    reduce is contiguous; a ones-column folds the softmax denominator
    into the same reduce.
  - One fused ACT copy drains V+K psum -> SBUF f16; score mul runs on
    DVE in f16 2x mode; amul on GpSimd; the three segment reduces stay
    on DVE (the only engine with free-axis reduce).
  - gate "cur" dot is folded into the q-projection matmul (8 extra
    columns) instead of a per-G vector pass.
  - all DMA descriptor issues moved to the Sync engine.
  - next-G-block PE transposes are software-pipelined one per phase.
  - per-molecule q-prepass is interleaved with the previous molecule.

Sharding: data-parallel over batch: 8 molecules per core x 8 cores.
"""

import sys

for _p in ("/opt/trn_rl_repo", "/root/.axon_site/_ro/trn_rl_repo"):
    if _p not in sys.path:
        sys.path.insert(0, _p)

from contextlib import ExitStack

import numpy as np

import concourse.bass as bass
import concourse.mybir as mybir
from concourse import bacc
from concourse.tile import TileContext

F16 = mybir.dt.float16
F32 = mybir.dt.float32
EXP = mybir.ActivationFunctionType.Exp
ADD = mybir.AluOpType.add
MAX = mybir.AluOpType.max
AXL_X = mybir.AxisListType.X

N_CORES = 8
BM = 8          # molecules per core
A = 128         # atoms
NEI = 10
D = 256
D2 = 512
NBLK = 2 * BM   # g-blocks per core (2 per molecule)


def _wa(r):
    return 256 - 64 * r


def _seg_ranges(r):
    """s-intervals of the 320-wide segment and their X^T d-chunk."""
    wa = _wa(r)
    return [(r, 0, wa), (r + 1, wa, 320)]


def _e_of(r, s):
    """weight-matrix column for segment position s of phase r."""
    wa = _wa(r)
    return 64 * r + s if s < wa else s - wa


DEBUG = False


def build_nc(with_bias: bool, bg_val: float) -> bass.Bass:
    nc = bacc.Bacc("TRN2", target_bir_lowering=False)

    dbg = {}
    if DEBUG:
        for nm, shp, dt in [
                ("dbg_qu16", [128, 4, 32], F16), ("dbg_qg", [128, 4], F32),
                ("dbg_kv16", [128, 2, 330], F16),
                ("dbg_smul", [128, 10, 32], F16),
                ("dbg_score", [128, 10], F32), ("dbg_ex", [128, 10], F16),
                ("dbg_amul", [128, 33, 10], F16),
                ("dbg_araw", [128, 4, 33], F32),
                ("dbg_emax", [128, 4, 32], F32),
                ("dbg_gave", [128, 4], F32), ("dbg_eg", [128, 4], F32),
                ("dbg_kvm", [128, 3, 320], F32),
                ("dbg_xt", [128, 128], F16)]:
            dbg[nm] = nc.declare_dram_parameter(nm, shp, dt, isOutput=True)

    x_h = nc.declare_dram_parameter("x", [BM, A * NEI, D2], F32, isOutput=False)
    qin_h = nc.declare_dram_parameter("qin", [BM, A, D], F32, isOutput=False)
    ident_h = nc.declare_dram_parameter("ident", [128, 128], F16, isOutput=False)
    wq_h = nc.declare_dram_parameter("wq", [128, 2, 264], F16, isOutput=False)
    wk_h = nc.declare_dram_parameter("wk", [128, 4, 256], F16, isOutput=False)
    wv_h = nc.declare_dram_parameter("wv", [128, 4, 4, 320], F16, isOutput=False)
    wm_h = nc.declare_dram_parameter("wm", [128, 4, 256], F16, isOutput=False)
    wgav_h = nc.declare_dram_parameter("wgav", [128, 1], F16, isOutput=False)
    if bg_val != 0.0:
        bgc_h = nc.declare_dram_parameter("bgc", [128, 1], F32, isOutput=False)
    wge_h = nc.declare_dram_parameter("wge", [128, 32], F32, isOutput=False)
    ssel_h = nc.declare_dram_parameter("ssel", [128, 32], F16, isOutput=False)
    s2sel_h = nc.declare_dram_parameter("s2sel", [32, 128], F16, isOutput=False)
    if with_bias:
        bkvm_h = nc.declare_dram_parameter("bkvm", [1, 3, 256], F16,
                                           isOutput=False)
        bvp_h = nc.declare_dram_parameter("bvp", [1, 4, 320], F16,
                                          isOutput=False)
        bq_h = nc.declare_dram_parameter("bq", [1, 264], F16, isOutput=False)
        ones_h = nc.declare_dram_parameter("ones", [1, 128], F16,
                                           isOutput=False)
    out_h = nc.declare_dram_parameter("out", [BM, A, D], F32, isOutput=True)

    # flat per-molecule views
    xg = (x_h[:].rearrange("b n c -> b (n c)")
          .rearrange("b (g p r t) -> b g p r t", g=2, p=128, r=4, t=640))
    o5 = (out_h[:].rearrange("b a c -> b (a c)")
          .rearrange("b (g p r k) -> b g p r k", g=2, p=128, r=4, k=32))

    with TileContext(nc) as tc, ExitStack() as ctx:
        consts = ctx.enter_context(tc.tile_pool(name="consts", bufs=1))
        sb_x = ctx.enter_context(tc.tile_pool(name="sbx", bufs=4))
        sb_xt = ctx.enter_context(tc.tile_pool(name="sbxt", bufs=9))
        sb_kv = ctx.enter_context(tc.tile_pool(name="sbkv", bufs=3))
        sb_ew = ctx.enter_context(tc.tile_pool(name="sbew", bufs=3))
        sb_g = ctx.enter_context(tc.tile_pool(name="sbg", bufs=3))
        sb_q = ctx.enter_context(tc.tile_pool(name="sbq", bufs=2))
        ps_p = ctx.enter_context(tc.tile_pool(name="pp", bufs=2, space="PSUM"))
        ps_t = ctx.enter_context(tc.tile_pool(name="pt", bufs=1, space="PSUM"))
        ps_m = ctx.enter_context(tc.tile_pool(name="pm", bufs=1, space="PSUM"))
        dram = ctx.enter_context(tc.tile_pool(name="dram", bufs=1,
                                              space="DRAM"))

        def cload(h, shape, dtype):
            t = consts.tile(shape, dtype, tag=h.name, name=h.name)
            nc.sync.dma_start(out=t, in_=h[:])
            return t

        # const load order matters at startup: transposes need ident, the
        # q prepass needs wq, phase r's matmuls need wk/wm and wv[r].
        ident_t = cload(ident_h, [128, 128], F16)
        wq_t = cload(wq_h, [128, 2, 264], F16)
        wgav_t = cload(wgav_h, [128, 1], F16)
        wv_t = consts.tile([128, 4, 4, 320], F16, tag="wv", name="wv")
        nc.sync.dma_start(out=wv_t[:, 0], in_=wv_h[:, 0])
        wk_t = cload(wk_h, [128, 4, 256], F16)
        nc.sync.dma_start(out=wv_t[:, 1], in_=wv_h[:, 1])
        wm_t = cload(wm_h, [128, 4, 256], F16)
        bgc_t = cload(bgc_h, [128, 1], F32) if bg_val != 0.0 else None
        wge_t = cload(wge_h, [128, 32], F32)
        ssel_t = cload(ssel_h, [128, 32], F16)
        s2sel_t = cload(s2sel_h, [32, 128], F16)
        if with_bias:
            bkvm_t = cload(bkvm_h, [1, 3, 256], F16)
            bvp_t = cload(bvp_h, [1, 4, 320], F16)
            bq_t = cload(bq_h, [1, 264], F16)
            ones_t = cload(ones_h, [1, 128], F16)

        qdram = dram.tile([BM, 128, 256], F16)
        qdramg = dram.tile([BM, 128, 8], F32)
        qin16s = {}

        def issue_qin(mol):
            t = sb_q.tile([128, 256], F16, tag="qin16", name="qin16",
                          bufs=BM)
            nc.gpsimd.dma_start(out=t, in_=qin_h[mol])
            qin16s[mol] = t

        # kv16 slot layout: [, 0, 0:10] ones (softmax denominator rides the
        # attention reduce), [, 0, 10:330] V, [, 0, 330] pg column,
        # [, 1, 10:330] K.  Ones are memset once per slot.
        KV_BUFS = 3
        for _ in range(KV_BUFS):
            kvi = sb_kv.tile([128, 2, 336], F16, tag="kv16", name="kv16",
                             bufs=KV_BUFS)
            nc.vector.memset(kvi[:, 0, 0:10], 1.0)

        def prepass(mol, skip_qin=False):
            """q/gate-cur projection for one molecule -> qdram[mol]."""
            if not skip_qin:
                issue_qin(mol)
            qin16 = qin16s.pop(mol)
            qtp = ps_t.tile([128, 2, 128], F16, tag="pt", name="qtp")
            for w in range(2):
                nc.tensor.transpose(qtp[:, w, :],
                                    qin16[:, 128 * w:128 * (w + 1)], ident_t)
            qT = sb_q.tile([128, 2, 128], F16, tag="qT", name="qT")
            nc.vector.tensor_copy(out=qT, in_=qtp)
            qpsum = ps_m.tile([128, 264], F32, tag="pm", name="qpsum")
            nc.tensor.matmul(qpsum, qT[:, 0, :], wq_t[:, 0, :],
                             start=True, stop=False)
            nc.tensor.matmul(qpsum, qT[:, 1, :], wq_t[:, 1, :],
                             start=False, stop=not with_bias)
            if with_bias:
                nc.tensor.matmul(qpsum, ones_t, bq_t, start=False, stop=True)
            qnat = sb_q.tile([128, 256], F16, tag="qnat", name="qnat")
            nc.scalar.copy(out=qnat, in_=qpsum[:, 0:256])
            nc.sync.dma_start(out=qdram[mol], in_=qnat)
            qng = sb_q.tile([128, 8], F32, tag="qng", name="qng")
            nc.scalar.copy(out=qng, in_=qpsum[:, 256:264])
            nc.sync.dma_start(out=qdramg[mol], in_=qng)

        # ---------------- per-block state ----------------
        xgt = {}      # block -> x16 tile [128, 4, 640]
        xts = {}      # block -> list of 4 XT sbuf tiles [128, 5, 128]
        qu16s = {}    # block -> [128, 4, 32] f16
        qgs = {}      # block -> [128, 4] f32
        gaveGs = {}   # block -> [128, 4] f32 (neighbor-mean gate dot)
        kvms = {}     # phase -> kvm psum tile
        kv16s = {}    # phase -> kv16 sbuf tile
        amuls = {}    # phase -> amul16 tile
        arawGs = {}   # block -> [128, 4, 33] f32 (col 0 = denominator)
        emaxGs = {}   # block -> [128, 4, 32] f32
        gtail = {}    # block -> dict of gate tiles

        def issue_x(bg, parts=1):
            """parts>1 splits the load by row-phase so the first transposes
            can start before the whole block has landed (startup only)."""
            mol, G = divmod(bg, 2)
            t = sb_x.tile([128, 4, 640], F16, tag="x16", name="x16")
            step = 4 // parts
            for r0 in range(0, 4, step):
                nc.gpsimd.dma_start(out=t[:, r0:r0 + step, :],
                                    in_=xg[mol, G, :, r0:r0 + step, :])
            xgt[bg] = t

        def issue_q(bg):
            # u-layout gather: partition p reads qdram row 64G + p//2,
            # column block 128*(p%2) -- two DMAs, one per parity.
            mol, G = divmod(bg, 2)
            qt = sb_ew.tile([128, 4, 32], F16, tag="qu16", name="qu16")
            gt = sb_g.tile([128, 4], F32, tag="qg", name="qg")
            qts = qt.rearrange("(p2 pb) r k -> pb p2 r k", pb=2)
            gts = gt.rearrange("(p2 pb) r -> pb p2 r", pb=2)
            for pb in range(2):
                src = (qdram[mol, 64 * G:64 * G + 64,
                             128 * pb:128 * pb + 128]
                       .rearrange("p (r k) -> p r k", r=4))
                nc.sync.dma_start(out=qts[pb], in_=src)
                nc.sync.dma_start(
                    out=gts[pb],
                    in_=qdramg[mol, 64 * G:64 * G + 64,
                               4 * pb:4 * pb + 4])
            qu16s[bg] = qt
            qgs[bg] = gt

        def t_batch(bg, r):
            """PE transposes of x block bg, Xu row-phase r + ACT copy."""
            tp = ps_t.tile([128, 5, 128], F16, tag="pt", name="tp")
            for w in range(5):
                nc.tensor.transpose(tp[:, w, :],
                                    xgt[bg][:, r, 128 * w:128 * (w + 1)],
                                    ident_t)
            xtb = sb_xt.tile([128, 5, 128], F16, tag="xt", name="xtb")
            nc.scalar.copy(out=xtb, in_=tp)
            if bg not in xts:
                xts[bg] = [None] * 4
            xts[bg][r] = xtb

        def XT(bg, d, fc):
            cid = 4 * d + fc
            return xts[bg][cid // 5][:, cid % 5, :]

        def phase_matmuls(i):
            bg, r = divmod(i, 4)
            kvm = ps_p.tile([128, 3, 512], F32, tag="pp", name="kvm")
            kvms[i] = kvm
            wa = _wa(r)
            nA = wa // 32
            # One accumulation group (start..stop) at a time per psum bank:
            # ranges outer, contraction chunks (fc) inner.
            # V: bank 0, (dk, nei)-packed columns.  The per-phase weight
            # pack stores each range's columns contiguously in (k, n)
            # enumeration order so the moving-operand fetch is sequential.
            vout = kvm[:, 0, 0:320].rearrange("p (k n) -> p k n", k=32)
            for (d, n0, n1) in ((r, 0, nA), (r + 1, nA, 10)):
                nr = n1 - n0
                wslc = (wv_t[:, r, :, 32 * n0:32 * n0 + 32 * nr]
                        .rearrange("p f (k n) -> p f k n", n=nr))
                for fc in range(4):
                    st, sp = fc == 0, (fc == 3) and not with_bias
                    nc.tensor.matmul(vout[:, :, n0:n1], XT(bg, d, fc),
                                     wslc[:, fc], start=st, stop=sp)
                if with_bias:
                    nc.tensor.matmul(
                        vout[:, :, n0:n1], ones_t,
                        bvp_t[:, r, 32 * n0:32 * n0 + 32 * nr]
                        .rearrange("o (k n) -> o k n", n=nr),
                        start=False, stop=True)
            # K: bank 1, segment order; weight col e0 = 64r (d=r) / 0 (d=r+1)
            wa = _wa(r)
            for (d, s0, s1) in _seg_ranges(r):
                e0 = 64 * r - s0 if d == r else -wa
                for fc in range(4):
                    st, sp = fc == 0, (fc == 3) and not with_bias
                    nc.tensor.matmul(kvm[:, 1, s0:s1], XT(bg, d, fc),
                                     wk_t[:, fc, s0 + e0:s1 + e0],
                                     start=st, stop=sp)
                if with_bias:
                    nc.tensor.matmul(kvm[:, 1, s0:s1], ones_t,
                                     bkvm_t[:, 1, s0 + e0:s1 + e0],
                                     start=False, stop=True)
            # M: bank 2, segment order
            for (d, s0, s1) in _seg_ranges(r):
                e0 = 64 * r - s0 if d == r else -wa
                for fc in range(4):
                    st, sp = fc == 0, (fc == 3) and not with_bias
                    nc.tensor.matmul(kvm[:, 2, s0:s1], XT(bg, d, fc),
                                     wm_t[:, fc, s0 + e0:s1 + e0],
                                     start=st, stop=sp)
                if with_bias:
                    nc.tensor.matmul(kvm[:, 2, s0:s1], ones_t,
                                     bkvm_t[:, 2, s0 + e0:s1 + e0],
                                     start=False, stop=True)
            # neighbor-mean gate dot, folded into the PE pass; accumulates
            # in the V bank's spare column (extracted with the kv copy)
            for j, cid in enumerate(range(5 * r, 5 * r + 5)):
                d, fc = divmod(cid, 4)
                nc.tensor.matmul(kvm[:, 0, 320:321], XT(bg, d, fc), wgav_t,
                                 start=(j == 0), stop=(j == 4),
                                 skip_group_check=True)

        def phase_front(i):
            """emax + kv drain + score chain for phase i (DVE/ACT)."""
            bg, r = divmod(i, 4)
            kvm = kvms[i]
            if bg not in emaxGs:
                emaxGs[bg] = sb_ew.tile([128, 4, 32], F32, tag="emaxG",
                                        name="emaxG", bufs=2)
                arawGs[bg] = sb_ew.tile([128, 4, 33], F32, tag="arawG",
                                        name="arawG", bufs=3)
            nc.vector.tensor_reduce(
                out=emaxGs[bg][:, r, :],
                in_=kvm[:, 2, 0:320].rearrange("p (j k) -> p k j", j=10),
                axis=AXL_X, op=MAX)

        def phase_score(i):
            bg, r = divmod(i, 4)
            kvm = kvms.pop(i)
            kv16 = sb_kv.tile([128, 2, 336], F16, tag="kv16", name="kv16",
                              bufs=KV_BUFS)
            kv16s[i] = kv16
            if DEBUG and i == 0:
                kvmc = sb_ew.tile([128, 3, 320], F32, tag="dbgkvm",
                                  name="kvmc", bufs=1)
                nc.vector.tensor_copy(out=kvmc, in_=kvm[:, :, 0:320])
                nc.sync.dma_start(out=dbg["dbg_kvm"][:], in_=kvmc)
            nc.scalar.copy(out=kv16[:, :, 10:331], in_=kvm[:, 0:2, 0:321])
            if bg not in gaveGs:
                gaveGs[bg] = sb_g.tile([128, 4], F32, tag="gaveG",
                                       name="gaveG", bufs=2)
            nc.gpsimd.tensor_copy(out=gaveGs[bg][:, r:r + 1],
                                  in_=kv16[:, 0, 330:331])
            smul = sb_ew.tile([128, 10, 32], F16, tag="smul", name="smul")
            nc.vector.tensor_mul(
                smul, kv16[:, 1, 10:330].rearrange("p (j k) -> p j k", j=10),
                qu16s[bg][:, r, :].unsqueeze(1).broadcast_to([128, 10, 32]))
            score = sb_ew.tile([128, 10], F32, tag="score", name="score")
            nc.vector.tensor_reduce(out=score, in_=smul, axis=AXL_X, op=ADD)
            ex = sb_ew.tile([128, 10], F16, tag="ex", name="ex")
            nc.scalar.activation(out=ex, in_=score, func=EXP)
            amul = sb_ew.tile([128, 33, 10], F16, tag="amul", name="amul")
            nc.gpsimd.tensor_mul(
                amul, kv16[:, 0, 0:330].rearrange("p (k n) -> p k n", k=33),
                ex.unsqueeze(1).broadcast_to([128, 33, 10]))
            amuls[i] = amul
            if DEBUG and i == 0:
                nc.sync.dma_start(out=dbg["dbg_kv16"][:], in_=kv16[:, :, 0:330])
                nc.sync.dma_start(out=dbg["dbg_smul"][:], in_=smul)
                nc.sync.dma_start(out=dbg["dbg_score"][:], in_=score)
                nc.sync.dma_start(out=dbg["dbg_ex"][:], in_=ex)
                nc.sync.dma_start(out=dbg["dbg_amul"][:], in_=amul)
                nc.sync.dma_start(out=dbg["dbg_qu16"][:], in_=qu16s[bg])
                nc.sync.dma_start(out=dbg["dbg_qg"][:], in_=qgs[bg])
                nc.sync.dma_start(out=dbg["dbg_xt"][:], in_=XT(bg, 0, 0))

        def phase_araw(i):
            bg, r = divmod(i, 4)
            nc.vector.tensor_reduce(out=arawGs[bg][:, r, :],
                                    in_=amuls.pop(i), axis=AXL_X, op=ADD)
            kv16s.pop(i)

        def g_tail_eg(bg):
            """gate logits for a finished g-block (independent of araw)."""
            emaxp = sb_g.tile([128, 4, 32], F32, tag="emaxp", name="emaxp",
                              bufs=2)
            nc.gpsimd.tensor_mul(
                emaxp, emaxGs[bg],
                wge_t.unsqueeze(1).broadcast_to([128, 4, 32]))
            gemxB = sb_g.tile([128, 4], F32, tag="gemxB", name="gemxB",
                              bufs=2)
            nc.vector.tensor_reduce(out=gemxB, in_=emaxp, axis=AXL_X, op=ADD)
            gl1 = sb_g.tile([128, 4], F32, tag="gl1", name="gl1", bufs=2)
            nc.gpsimd.tensor_add(gl1, qgs.pop(bg), gemxB)
            gl2 = sb_g.tile([128, 4], F32, tag="gl2", name="gl2", bufs=2)
            nc.gpsimd.tensor_add(gl2, gl1, gaveGs[bg])
            egB = sb_g.tile([128, 4], F32, tag="egB", name="egB", bufs=3)
            if bg_val != 0.0:
                nc.scalar.activation(out=egB, in_=gl2, func=EXP, bias=bgc_t)
            else:
                nc.scalar.activation(out=egB, in_=gl2, func=EXP)
            eg16 = sb_g.tile([128, 4], F16, tag="eg16", name="eg16", bufs=3)
            nc.vector.tensor_copy(out=eg16, in_=egB)
            gtail[bg] = {"egB": egB, "eg16": eg16}
            if DEBUG and bg == 0:
                nc.sync.dma_start(out=dbg["dbg_emax"][:], in_=emaxGs[bg])
                nc.sync.dma_start(out=dbg["dbg_gave"][:], in_=gaveGs[bg])
                nc.sync.dma_start(out=dbg["dbg_eg"][:], in_=egB)

        def g_tail_ra(bg):
            raB = sb_g.tile([128, 4], F32, tag="raB", name="raB", bufs=3)
            nc.vector.reciprocal(out=raB, in_=arawGs[bg][:, :, 0])
            gtail[bg]["raB"] = raB
            if DEBUG and bg == 0:
                nc.sync.dma_start(out=dbg["dbg_araw"][:], in_=arawGs[bg])

        def mol_tail_a(mol):
            """head-sum of gate numerators (PE) + reciprocal."""
            g0, g1 = gtail[2 * mol], gtail[2 * mol + 1]
            gd = ps_m.tile([32, 4], F32, tag="pm", name="gd")
            for r in range(4):
                nc.tensor.matmul(gd[:, r:r + 1], ssel_t,
                                 g0["eg16"][:, r:r + 1],
                                 start=True, stop=False)
                nc.tensor.matmul(gd[:, r:r + 1], ssel_t,
                                 g1["eg16"][:, r:r + 1],
                                 start=False, stop=True)
            rg = sb_g.tile([32, 4], F32, tag="rg", name="rg", bufs=2)
            nc.vector.reciprocal(out=rg, in_=gd)
            rg16 = sb_g.tile([32, 4], F16, tag="rg16", name="rg16", bufs=2)
            nc.vector.tensor_copy(out=rg16, in_=rg)
            return rg16

        def mol_tail_b(mol, rg16):
            """broadcast the head-sum back and scale the raw attention."""
            g0, g1 = gtail.pop(2 * mol), gtail.pop(2 * mol + 1)
            inv = ps_m.tile([128, 4], F32, tag="pm", name="inv")
            for r in range(4):
                nc.tensor.matmul(inv[:, r:r + 1], s2sel_t, rg16[:, r:r + 1],
                                 start=True, stop=True)
            invs = sb_g.tile([128, 4], F32, tag="invs", name="invs", bufs=2)
            nc.scalar.copy(out=invs, in_=inv)
            for gg, gt in ((0, g0), (1, g1)):
                bg = 2 * mol + gg
                t1 = sb_g.tile([128, 4], F32, tag="t1", name="t1", bufs=2)
                nc.gpsimd.tensor_mul(t1, invs, gt["raB"])
                c2 = sb_g.tile([128, 4], F32, tag="c2", name="c2", bufs=2)
                nc.gpsimd.tensor_mul(c2, t1, gt["egB"])
                outB = sb_g.tile([128, 4, 32], F32, tag="outB", name="outB",
                                 bufs=4)
                nc.gpsimd.tensor_mul(
                    outB, arawGs.pop(bg)[:, :, 1:33],
                    c2.unsqueeze(2).broadcast_to([128, 4, 32]))
                nc.sync.dma_start(out=o5[mol, gg], in_=outB)
                emaxGs.pop(bg, None)
                gaveGs.pop(bg, None)

        # ---------------- prologue ----------------
        # The cast ring (gpsimd) carries the first two qin loads then the
        # first two blocks' x parts, interleaved so transposes start early.
        issue_qin(0)
        xgt[0] = sb_x.tile([128, 4, 640], F16, tag="x16", name="x16")
        xgt[1] = sb_x.tile([128, 4, 640], F16, tag="x16", name="x16")
        xparts = [(0, 0), (1, 0)]
        issue_qin(1)
        xparts += [(0, 1), (2, 0), (1, 1), (3, 0), (2, 1), (3, 1)]
        for rr, bb in xparts:
            mol_, G_ = divmod(bb, 2)
            nc.gpsimd.dma_start(out=xgt[bb][:, rr:rr + 1, :],
                                in_=xg[mol_, G_, :, rr:rr + 1, :])
        for m in range(2, BM):
            issue_qin(m)
        prepass(0, skip_qin=True)
        prepass(1, skip_qin=True)
        issue_q(0)
        nc.sync.dma_start(out=wv_t[:, 2], in_=wv_h[:, 2])
        nc.sync.dma_start(out=wv_t[:, 3], in_=wv_h[:, 3])
        for r in range(4):
            t_batch(0, r)

        NPH = 4 * NBLK
        pend_b = {}                     # stage idx -> (mol, rg16)
        for i in range(NPH):
            bg, r = divmod(i, 4)
            mol, G = divmod(bg, 2)
            if r == 0 and bg + 2 < NBLK:
                issue_x(bg + 2)
            # PE: transposes for phase i+4's block, one batch per phase
            if i + 4 < NPH:
                bg2, r2 = divmod(i + 4, 4)
                t_batch(bg2, r2)
            if i > 0 and r == 0:
                g_tail_eg(bg - 1)      # independent of araw; feeds gd early
            if i > 0:
                phase_araw(i - 1)
                if r == 0:
                    g_tail_ra(bg - 1)
            phase_matmuls(i)
            if i > 0 and r == 0 and G == 0 and mol > 0:
                pend_b[i + 1] = (mol - 1, mol_tail_a(mol - 1))
            if i in pend_b:             # PE inv one stage after gd
                m_, rg16_ = pend_b.pop(i)
                mol_tail_b(m_, rg16_)
            phase_front(i)
            phase_score(i)
            if i >= 4 and i % 2 == 0 and i // 2 < BM:
                prepass(i // 2, skip_qin=True)   # prepasses 2..7 interleaved
            if r == 1 and bg + 1 < NBLK:
                issue_q(bg + 1)
        phase_araw(NPH - 1)
        g_tail_eg(NBLK - 1)
        g_tail_ra(NBLK - 1)
        mol_tail_b(BM - 1, mol_tail_a(BM - 1))

    nc.finalize()
    return nc


def _prep_consts(Wq, bq, Wk, bk, Wv, bv, Wam, bam, Wg, bg):
    Wq = np.asarray(Wq, np.float32)
    Wk = np.asarray(Wk, np.float32)
    Wv = np.asarray(Wv, np.float32)
    Wam = np.asarray(Wam, np.float32)
    wg = np.asarray(Wg, np.float32).reshape(-1)

    # q projection + gate-cur block-diagonal columns
    wq = np.zeros((128, 2, 264), np.float32)
    for c in range(2):
        wq[:, c, 0:256] = Wq[128 * c:128 * (c + 1), :]
        for p in range(128):
            ch = 128 * c + p
            wq[p, c, 256 + ch // 32] = wg[ch % 32]

    # K/M natural packs (row-chunked): pack[p, fc, c] = W[128*fc + p, c]
    wk_c = np.empty((128, 4, 256), np.float32)
    wm_c = np.empty((128, 4, 256), np.float32)
    for fc in range(4):
        rows = slice(128 * fc, 128 * (fc + 1))
        wk_c[:, fc, :] = Wk[rows, :]
        wm_c[:, fc, :] = Wam[rows, :]
    # V per-phase pack, each range's columns contiguous in (k, n) order
    wv_c = np.empty((128, 4, 4, 320), np.float32)
    for r in range(4):
        wa = 256 - 64 * r
        nA = wa // 32
        cols = np.empty((32, NEI), np.int64)
        for k in range(32):
            for n in range(NEI):
                sv = 32 * n + k
                cols[k, n] = 64 * r + sv if sv < wa else sv - wa
        order = np.concatenate([cols[:, 0:nA].reshape(-1),
                                cols[:, nA:NEI].reshape(-1)])
        for fc in range(4):
            rows = slice(128 * fc, 128 * (fc + 1))
            wv_c[:, r, fc, :] = Wv[rows, :][:, order]

    p = np.arange(128)
    ssel = (p[:, None] % 32 == np.arange(32)[None, :]).astype(np.float16)
    consts = {
        "ident": np.eye(128, dtype=np.float16),
        "wq": wq.astype(np.float16),
        "wk": wk_c.astype(np.float16),
        "wv": wv_c.astype(np.float16),
        "wm": wm_c.astype(np.float16),
        "wgav": (wg[64 + (p % 64)] / NEI).astype(np.float16).reshape(128, 1),
        "wge": np.tile(wg[32:64], (128, 1)).astype(np.float32),
        "ssel": ssel,
        "s2sel": ssel.T.copy(),
    }
    bg_val = float(np.asarray(bg).reshape(-1)[0])
    if bg_val != 0.0:
        consts["bgc"] = np.full((128, 1), bg_val, np.float32)
    with_bias = any(np.any(np.asarray(b) != 0) for b in (bq, bk, bv, bam))
    if with_bias:
        bkvm = np.stack([np.asarray(bv, np.float32),
                         np.asarray(bk, np.float32),
                         np.asarray(bam, np.float32)])[None]
        consts["bkvm"] = bkvm.astype(np.float16)
        bvp = np.empty((1, 4, 320), np.float32)
        bvf = np.asarray(bv, np.float32)
        for r in range(4):
            wa = 256 - 64 * r
            nA = wa // 32
            cols = np.empty((32, NEI), np.int64)
            for k in range(32):
                for n in range(NEI):
                    sv = 32 * n + k
                    cols[k, n] = 64 * r + sv if sv < wa else sv - wa
            order = np.concatenate([cols[:, 0:nA].reshape(-1),
                                    cols[:, nA:NEI].reshape(-1)])
            bvp[0, r, :] = bvf[order]
        consts["bvp"] = bvp.astype(np.float16)
        bqp = np.zeros((1, 264), np.float32)
        bqp[0, 0:256] = np.asarray(bq, np.float32)
        consts["bq"] = bqp.astype(np.float16)
        consts["ones"] = np.ones((1, 128), np.float16)
    return consts, with_bias, float(np.asarray(bg).reshape(-1)[0])


_CACHE = {}
TRACE = False       # set by test.py for profiling runs
LAST_RESULTS = None  # BassKernelResults from the most recent run


def kernel(input_multihead, input_q, Wq, bq, Wk, bk, Wv, bv, Wam, bam, Wg, bg):
    from concourse.bass_utils import run_bass_kernel_spmd

    consts, with_bias, bg_val = _prep_consts(
        Wq, bq, Wk, bk, Wv, bv, Wam, bam, Wg, bg)

    key = (with_bias, bg_val, DEBUG)
    if key not in _CACHE:
        _CACHE[key] = build_nc(with_bias, bg_val)
    nc = _CACHE[key]

    x = np.ascontiguousarray(np.asarray(input_multihead, np.float32))
    q = np.ascontiguousarray(np.asarray(input_q, np.float32))
    in_maps = []
    for c in range(N_CORES):
        m = {"x": x[BM * c:BM * (c + 1)], "qin": q[BM * c:BM * (c + 1)]}
        m.update(consts)
        in_maps.append(m)

    res = run_bass_kernel_spmd(nc, in_maps, list(range(N_CORES)), trace=TRACE)
    global LAST_RESULTS
    LAST_RESULTS = res
    return np.concatenate([res.results[c]["out"] for c in range(N_CORES)],
                          axis=0)


# revision 70
# speedup vs baseline: 1.1259x; 1.0200x over previous
"""Trainium2 Bass kernel for nn_MultiHeadedAttentionWithGate (v2).

Math (per molecule, validated against reference): the reference's
reshapes are flat views, so with u = "virtual row" (1024 per molecule),
the computation is per-u over contiguous flat segments: K/V/M rows of
320 (10 nei x 32), X rows of 640 (10 x 64), q rows of 32.

Phase decomposition: u = 4*g + r.  For fixed phase r (0..3) and g on
partitions, every tensor's u-row is a contiguous DRAM segment and the
K/V/M[u-layout] projections decompose into matmuls over X^T chunks
(the f16 PE-transposes of the per-phase Xu tiles chunked by 128 cols).

v2 schedule (vs v1):
  - V psum columns packed (dk, nei)-major so the attention-weighted
    reduce is contiguous; a ones-column folds the softmax denominator
    into the same reduce.
  - One fused ACT copy drains V+K psum -> SBUF f16; score mul runs on
    DVE in f16 2x mode; amul on GpSimd; the three segment reduces stay
    on DVE (the only engine with free-axis reduce).
  - gate "cur" dot is folded into the q-projection matmul (8 extra
    columns) instead of a per-G vector pass.
  - all DMA descriptor issues moved to the Sync engine.
  - next-G-block PE transposes are software-pipelined one per phase.
  - per-molecule q-prepass is interleaved with the previous molecule.

Sharding: data-parallel over batch: 8 molecules per core x 8 cores.
"""

import sys

for _p in ("/opt/trn_rl_repo", "/root/.axon_site/_ro/trn_rl_repo"):
    if _p not in sys.path:
        sys.path.insert(0, _p)

from contextlib import ExitStack

import numpy as np

import concourse.bass as bass
import concourse.mybir as mybir
from concourse import bacc
from concourse.tile import TileContext

F16 = mybir.dt.float16
F32 = mybir.dt.float32
EXP = mybir.ActivationFunctionType.Exp
ADD = mybir.AluOpType.add
MAX = mybir.AluOpType.max
AXL_X = mybir.AxisListType.X

N_CORES = 8
BM = 8          # molecules per core
A = 128         # atoms
NEI = 10
D = 256
D2 = 512
NBLK = 2 * BM   # g-blocks per core (2 per molecule)


def _wa(r):
    return 256 - 64 * r


def _seg_ranges(r):
    """s-intervals of the 320-wide segment and their X^T d-chunk."""
    wa = _wa(r)
    return [(r, 0, wa), (r + 1, wa, 320)]


def _e_of(r, s):
    """weight-matrix column for segment position s of phase r."""
    wa = _wa(r)
    return 64 * r + s if s < wa else s - wa


DEBUG = False


def build_nc(with_bias: bool, bg_val: float) -> bass.Bass:
    nc = bacc.Bacc("TRN2", target_bir_lowering=False)

    dbg = {}
    if DEBUG:
        for nm, shp, dt in [
                ("dbg_qu16", [128, 4, 32], F16), ("dbg_qg", [128, 4], F32),
                ("dbg_kv16", [128, 2, 330], F16),
                ("dbg_smul", [128, 10, 32], F16),
                ("dbg_score", [128, 10], F32), ("dbg_ex", [128, 10], F16),
                ("dbg_amul", [128, 33, 10], F16),
                ("dbg_araw", [128, 4, 33], F32),
                ("dbg_emax", [128, 4, 32], F32),
                ("dbg_gave", [128, 4], F32), ("dbg_eg", [128, 4], F32),
                ("dbg_kvm", [128, 3, 320], F32),
                ("dbg_xt", [128, 128], F16)]:
            dbg[nm] = nc.declare_dram_parameter(nm, shp, dt, isOutput=True)

    x_h = nc.declare_dram_parameter("x", [BM, A * NEI, D2], F32, isOutput=False)
    qin_h = nc.declare_dram_parameter("qin", [BM, A, D], F32, isOutput=False)
    ident_h = nc.declare_dram_parameter("ident", [128, 128], F16, isOutput=False)
    wq_h = nc.declare_dram_parameter("wq", [128, 2, 264], F16, isOutput=False)
    wk_h = nc.declare_dram_parameter("wk", [128, 4, 256], F16, isOutput=False)
    wv_h = nc.declare_dram_parameter("wv", [128, 4, 4, 320], F16, isOutput=False)
    wm_h = nc.declare_dram_parameter("wm", [128, 4, 256], F16, isOutput=False)
    wgav_h = nc.declare_dram_parameter("wgav", [128, 1], F16, isOutput=False)
    if bg_val != 0.0:
        bgc_h = nc.declare_dram_parameter("bgc", [128, 1], F32, isOutput=False)
    wge_h = nc.declare_dram_parameter("wge", [128, 32], F32, isOutput=False)
    ssel_h = nc.declare_dram_parameter("ssel", [128, 32], F16, isOutput=False)
    s2sel_h = nc.declare_dram_parameter("s2sel", [32, 128], F16, isOutput=False)
    if with_bias:
        bkvm_h = nc.declare_dram_parameter("bkvm", [1, 3, 256], F16,
                                           isOutput=False)
        bvp_h = nc.declare_dram_parameter("bvp", [1, 4, 320], F16,
                                          isOutput=False)
        bq_h = nc.declare_dram_parameter("bq", [1, 264], F16, isOutput=False)
        ones_h = nc.declare_dram_parameter("ones", [1, 128], F16,
                                           isOutput=False)
    out_h = nc.declare_dram_parameter("out", [BM, A, D], F32, isOutput=True)

    # flat per-molecule views
    xg = (x_h[:].rearrange("b n c -> b (n c)")
          .rearrange("b (g p r t) -> b g p r t", g=2, p=128, r=4, t=640))
    o5 = (out_h[:].rearrange("b a c -> b (a c)")
          .rearrange("b (g p r k) -> b g p r k", g=2, p=128, r=4, k=32))

    with TileContext(nc) as tc, ExitStack() as ctx:
        consts = ctx.enter_context(tc.tile_pool(name="consts", bufs=1))
        sb_x = ctx.enter_context(tc.tile_pool(name="sbx", bufs=4))
        sb_xt = ctx.enter_context(tc.tile_pool(name="sbxt", bufs=9))
        sb_kv = ctx.enter_context(tc.tile_pool(name="sbkv", bufs=3))
        sb_ew = ctx.enter_context(tc.tile_pool(name="sbew", bufs=3))
        sb_g = ctx.enter_context(tc.tile_pool(name="sbg", bufs=3))
        sb_q = ctx.enter_context(tc.tile_pool(name="sbq", bufs=2))
        ps_p = ctx.enter_context(tc.tile_pool(name="pp", bufs=2, space="PSUM"))
        ps_t = ctx.enter_context(tc.tile_pool(name="pt", bufs=1, space="PSUM"))
        ps_m = ctx.enter_context(tc.tile_pool(name="pm", bufs=1, space="PSUM"))
        dram = ctx.enter_context(tc.tile_pool(name="dram", bufs=1,
                                              space="DRAM"))

        def cload(h, shape, dtype):
            t = consts.tile(shape, dtype, tag=h.name, name=h.name)
            nc.sync.dma_start(out=t, in_=h[:])
            return t

        # const load order matters at startup: transposes need ident, the
        # q prepass needs wq, phase r's matmuls need wk/wm and wv[r].
        ident_t = cload(ident_h, [128, 128], F16)
        wq_t = cload(wq_h, [128, 2, 264], F16)
        wgav_t = cload(wgav_h, [128, 1], F16)
        wv_t = consts.tile([128, 4, 4, 320], F16, tag="wv", name="wv")
        wk_t = consts.tile([128, 4, 256], F16, tag="wk", name="wk")
        wm_t = consts.tile([128, 4, 256], F16, tag="wm", name="wm")
        bgc_t = (consts.tile([128, 1], F32, tag="bgc", name="bgc")
                 if bg_val != 0.0 else None)
        wge_t = consts.tile([128, 32], F32, tag="wge", name="wge")
        ssel_t = consts.tile([128, 32], F16, tag="ssel", name="ssel")
        s2sel_t = consts.tile([32, 128], F16, tag="s2sel", name="s2sel")

        def load_main_consts():
            nc.sync.dma_start(out=wv_t[:, 0], in_=wv_h[:, 0])
            nc.sync.dma_start(out=wk_t, in_=wk_h[:])
            nc.sync.dma_start(out=wv_t[:, 1], in_=wv_h[:, 1])
            nc.sync.dma_start(out=wm_t, in_=wm_h[:])
            if bg_val != 0.0:
                nc.sync.dma_start(out=bgc_t, in_=bgc_h[:])
            nc.sync.dma_start(out=wge_t, in_=wge_h[:])
            nc.sync.dma_start(out=ssel_t, in_=ssel_h[:])
            nc.sync.dma_start(out=s2sel_t, in_=s2sel_h[:])
        if with_bias:
            bkvm_t = cload(bkvm_h, [1, 3, 256], F16)
            bvp_t = cload(bvp_h, [1, 4, 320], F16)
            bq_t = cload(bq_h, [1, 264], F16)
            ones_t = cload(ones_h, [1, 128], F16)

        qdram = dram.tile([BM, 128, 256], F16)
        qdramg = dram.tile([BM, 128, 8], F32)
        qin32s = {}

        def issue_qin(mol):
            t = sb_q.tile([128, 256], F32, tag="qin32", name="qin32",
                          bufs=BM)
            nc.sync.dma_start(out=t, in_=qin_h[mol])
            qin32s[mol] = t

        # kv16 slot layout: [, 0, 0:10] ones (softmax denominator rides the
        # attention reduce), [, 0, 10:330] V, [, 0, 330] pg column,
        # [, 1, 10:330] K.  Ones are memset once per slot.
        KV_BUFS = 3
        for _ in range(KV_BUFS):
            kvi = sb_kv.tile([128, 2, 336], F16, tag="kv16", name="kv16",
                             bufs=KV_BUFS)
            nc.vector.memset(kvi[:, 0, 0:10], 1.0)

        def prepass(mol, skip_qin=False):
            """q/gate-cur projection for one molecule -> qdram[mol]."""
            if not skip_qin:
                issue_qin(mol)
            qin16 = sb_q.tile([128, 256], F16, tag="qin16", name="qin16")
            nc.vector.tensor_copy(out=qin16, in_=qin32s.pop(mol))
            qtp = ps_t.tile([128, 2, 128], F16, tag="pt", name="qtp")
            for w in range(2):
                nc.tensor.transpose(qtp[:, w, :],
                                    qin16[:, 128 * w:128 * (w + 1)], ident_t)
            qT = sb_q.tile([128, 2, 128], F16, tag="qT", name="qT")
            nc.vector.tensor_copy(out=qT, in_=qtp)
            qpsum = ps_m.tile([128, 264], F32, tag="pm", name="qpsum")
            nc.tensor.matmul(qpsum, qT[:, 0, :], wq_t[:, 0, :],
                             start=True, stop=False)
            nc.tensor.matmul(qpsum, qT[:, 1, :], wq_t[:, 1, :],
                             start=False, stop=not with_bias)
            if with_bias:
                nc.tensor.matmul(qpsum, ones_t, bq_t, start=False, stop=True)
            qnat = sb_q.tile([128, 256], F16, tag="qnat", name="qnat")
            nc.scalar.copy(out=qnat, in_=qpsum[:, 0:256])
            nc.sync.dma_start(out=qdram[mol], in_=qnat)
            qng = sb_q.tile([128, 8], F32, tag="qng", name="qng")
            nc.scalar.copy(out=qng, in_=qpsum[:, 256:264])
            nc.sync.dma_start(out=qdramg[mol], in_=qng)

        # ---------------- per-block state ----------------
        xgt = {}      # block -> x16 tile [128, 4, 640]
        xts = {}      # block -> list of 4 XT sbuf tiles [128, 5, 128]
        qu16s = {}    # block -> [128, 4, 32] f16
        qgs = {}      # block -> [128, 4] f32
        gaveGs = {}   # block -> [128, 4] f32 (neighbor-mean gate dot)
        kvms = {}     # phase -> kvm psum tile
        kv16s = {}    # phase -> kv16 sbuf tile
        amuls = {}    # phase -> amul16 tile
        arawGs = {}   # block -> [128, 4, 33] f32 (col 0 = denominator)
        emaxGs = {}   # block -> [128, 4, 32] f32
        gtail = {}    # block -> dict of gate tiles

        def issue_x(bg, parts=1):
            """parts>1 splits the load by row-phase so the first transposes
            can start before the whole block has landed (startup only)."""
            mol, G = divmod(bg, 2)
            t = sb_x.tile([128, 4, 640], F16, tag="x16", name="x16")
            step = 4 // parts
            for r0 in range(0, 4, step):
                nc.gpsimd.dma_start(out=t[:, r0:r0 + step, :],
                                    in_=xg[mol, G, :, r0:r0 + step, :])
            xgt[bg] = t

        def issue_q(bg):
            # u-layout gather: partition p reads qdram row 64G + p//2,
            # column block 128*(p%2) -- two DMAs, one per parity.
            mol, G = divmod(bg, 2)
            qt = sb_ew.tile([128, 4, 32], F16, tag="qu16", name="qu16")
            gt = sb_g.tile([128, 4], F32, tag="qg", name="qg")
            qts = qt.rearrange("(p2 pb) r k -> pb p2 r k", pb=2)
            gts = gt.rearrange("(p2 pb) r -> pb p2 r", pb=2)
            for pb in range(2):
                src = (qdram[mol, 64 * G:64 * G + 64,
                             128 * pb:128 * pb + 128]
                       .rearrange("p (r k) -> p r k", r=4))
                nc.sync.dma_start(out=qts[pb], in_=src)
                nc.sync.dma_start(
                    out=gts[pb],
                    in_=qdramg[mol, 64 * G:64 * G + 64,
                               4 * pb:4 * pb + 4])
            qu16s[bg] = qt
            qgs[bg] = gt

        def t_batch(bg, r):
            """PE transposes of x block bg, Xu row-phase r + ACT copy."""
            tp = ps_t.tile([128, 5, 128], F16, tag="pt", name="tp")
            for w in range(5):
                nc.tensor.transpose(tp[:, w, :],
                                    xgt[bg][:, r, 128 * w:128 * (w + 1)],
                                    ident_t)
            xtb = sb_xt.tile([128, 5, 128], F16, tag="xt", name="xtb")
            nc.scalar.copy(out=xtb, in_=tp)
            if bg not in xts:
                xts[bg] = [None] * 4
            xts[bg][r] = xtb

        def XT(bg, d, fc):
            cid = 4 * d + fc
            return xts[bg][cid // 5][:, cid % 5, :]

        def phase_matmuls(i):
            bg, r = divmod(i, 4)
            kvm = ps_p.tile([128, 3, 512], F32, tag="pp", name="kvm")
            kvms[i] = kvm
            wa = _wa(r)
            nA = wa // 32
            # One accumulation group (start..stop) at a time per psum bank:
            # ranges outer, contraction chunks (fc) inner.
            # V: bank 0, (dk, nei)-packed columns.  The per-phase weight
            # pack stores each range's columns contiguously in (k, n)
            # enumeration order so the moving-operand fetch is sequential.
            vout = kvm[:, 0, 0:320].rearrange("p (k n) -> p k n", k=32)
            for (d, n0, n1) in ((r, 0, nA), (r + 1, nA, 10)):
                nr = n1 - n0
                wslc = (wv_t[:, r, :, 32 * n0:32 * n0 + 32 * nr]
                        .rearrange("p f (k n) -> p f k n", n=nr))
                for fc in range(4):
                    st, sp = fc == 0, (fc == 3) and not with_bias
                    nc.tensor.matmul(vout[:, :, n0:n1], XT(bg, d, fc),
                                     wslc[:, fc], start=st, stop=sp)
                if with_bias:
                    nc.tensor.matmul(
                        vout[:, :, n0:n1], ones_t,
                        bvp_t[:, r, 32 * n0:32 * n0 + 32 * nr]
                        .rearrange("o (k n) -> o k n", n=nr),
                        start=False, stop=True)
            # K: bank 1, segment order; weight col e0 = 64r (d=r) / 0 (d=r+1)
            wa = _wa(r)
            for (d, s0, s1) in _seg_ranges(r):
                e0 = 64 * r - s0 if d == r else -wa
                for fc in range(4):
                    st, sp = fc == 0, (fc == 3) and not with_bias
                    nc.tensor.matmul(kvm[:, 1, s0:s1], XT(bg, d, fc),
                                     wk_t[:, fc, s0 + e0:s1 + e0],
                                     start=st, stop=sp)
                if with_bias:
                    nc.tensor.matmul(kvm[:, 1, s0:s1], ones_t,
                                     bkvm_t[:, 1, s0 + e0:s1 + e0],
                                     start=False, stop=True)
            # M: bank 2, segment order
            for (d, s0, s1) in _seg_ranges(r):
                e0 = 64 * r - s0 if d == r else -wa
                for fc in range(4):
                    st, sp = fc == 0, (fc == 3) and not with_bias
                    nc.tensor.matmul(kvm[:, 2, s0:s1], XT(bg, d, fc),
                                     wm_t[:, fc, s0 + e0:s1 + e0],
                                     start=st, stop=sp)
                if with_bias:
                    nc.tensor.matmul(kvm[:, 2, s0:s1], ones_t,
                                     bkvm_t[:, 2, s0 + e0:s1 + e0],
                                     start=False, stop=True)
            # neighbor-mean gate dot, folded into the PE pass; accumulates
            # in the V bank's spare column (extracted with the kv copy)
            for j, cid in enumerate(range(5 * r, 5 * r + 5)):
                d, fc = divmod(cid, 4)
                nc.tensor.matmul(kvm[:, 0, 320:321], XT(bg, d, fc), wgav_t,
                                 start=(j == 0), stop=(j == 4),
                                 skip_group_check=True)

        def phase_front(i):
            """emax + kv drain + score chain for phase i (DVE/ACT)."""
            bg, r = divmod(i, 4)
            kvm = kvms[i]
            if bg not in emaxGs:
                emaxGs[bg] = sb_ew.tile([128, 4, 32], F32, tag="emaxG",
                                        name="emaxG", bufs=2)
                arawGs[bg] = sb_ew.tile([128, 4, 33], F32, tag="arawG",
                                        name="arawG", bufs=3)
            nc.vector.tensor_reduce(
                out=emaxGs[bg][:, r, :],
                in_=kvm[:, 2, 0:320].rearrange("p (j k) -> p k j", j=10),
                axis=AXL_X, op=MAX)

        def phase_score(i):
            bg, r = divmod(i, 4)
            kvm = kvms.pop(i)
            kv16 = sb_kv.tile([128, 2, 336], F16, tag="kv16", name="kv16",
                              bufs=KV_BUFS)
            kv16s[i] = kv16
            if DEBUG and i == 0:
                kvmc = sb_ew.tile([128, 3, 320], F32, tag="dbgkvm",
                                  name="kvmc", bufs=1)
                nc.vector.tensor_copy(out=kvmc, in_=kvm[:, :, 0:320])
                nc.sync.dma_start(out=dbg["dbg_kvm"][:], in_=kvmc)
            nc.scalar.copy(out=kv16[:, :, 10:331], in_=kvm[:, 0:2, 0:321])
            if bg not in gaveGs:
                gaveGs[bg] = sb_g.tile([128, 4], F32, tag="gaveG",
                                       name="gaveG", bufs=2)
            nc.gpsimd.tensor_copy(out=gaveGs[bg][:, r:r + 1],
                                  in_=kv16[:, 0, 330:331])
            smul = sb_ew.tile([128, 10, 32], F16, tag="smul", name="smul")
            nc.vector.tensor_mul(
                smul, kv16[:, 1, 10:330].rearrange("p (j k) -> p j k", j=10),
                qu16s[bg][:, r, :].unsqueeze(1).broadcast_to([128, 10, 32]))
            score = sb_ew.tile([128, 10], F32, tag="score", name="score")
            nc.vector.tensor_reduce(out=score, in_=smul, axis=AXL_X, op=ADD)
            ex = sb_ew.tile([128, 10], F16, tag="ex", name="ex")
            nc.scalar.activation(out=ex, in_=score, func=EXP)
            amul = sb_ew.tile([128, 33, 10], F16, tag="amul", name="amul")
            nc.gpsimd.tensor_mul(
                amul, kv16[:, 0, 0:330].rearrange("p (k n) -> p k n", k=33),
                ex.unsqueeze(1).broadcast_to([128, 33, 10]))
            amuls[i] = amul
            if DEBUG and i == 0:
                nc.sync.dma_start(out=dbg["dbg_kv16"][:], in_=kv16[:, :, 0:330])
                nc.sync.dma_start(out=dbg["dbg_smul"][:], in_=smul)
                nc.sync.dma_start(out=dbg["dbg_score"][:], in_=score)
                nc.sync.dma_start(out=dbg["dbg_ex"][:], in_=ex)
                nc.sync.dma_start(out=dbg["dbg_amul"][:], in_=amul)
                nc.sync.dma_start(out=dbg["dbg_qu16"][:], in_=qu16s[bg])
                nc.sync.dma_start(out=dbg["dbg_qg"][:], in_=qgs[bg])
                nc.sync.dma_start(out=dbg["dbg_xt"][:], in_=XT(bg, 0, 0))

        def phase_araw(i):
            bg, r = divmod(i, 4)
            nc.vector.tensor_reduce(out=arawGs[bg][:, r, :],
                                    in_=amuls.pop(i), axis=AXL_X, op=ADD)
            kv16s.pop(i)

        def g_tail_eg(bg):
            """gate logits for a finished g-block (independent of araw)."""
            emaxp = sb_g.tile([128, 4, 32], F32, tag="emaxp", name="emaxp",
                              bufs=2)
            nc.gpsimd.tensor_mul(
                emaxp, emaxGs[bg],
                wge_t.unsqueeze(1).broadcast_to([128, 4, 32]))
            gemxB = sb_g.tile([128, 4], F32, tag="gemxB", name="gemxB",
                              bufs=2)
            nc.vector.tensor_reduce(out=gemxB, in_=emaxp, axis=AXL_X, op=ADD)
            gl1 = sb_g.tile([128, 4], F32, tag="gl1", name="gl1", bufs=2)
            nc.gpsimd.tensor_add(gl1, qgs.pop(bg), gemxB)
            gl2 = sb_g.tile([128, 4], F32, tag="gl2", name="gl2", bufs=2)
            nc.gpsimd.tensor_add(gl2, gl1, gaveGs[bg])
            egB = sb_g.tile([128, 4], F32, tag="egB", name="egB", bufs=3)
            if bg_val != 0.0:
                nc.scalar.activation(out=egB, in_=gl2, func=EXP, bias=bgc_t)
            else:
                nc.scalar.activation(out=egB, in_=gl2, func=EXP)
            eg16 = sb_g.tile([128, 4], F16, tag="eg16", name="eg16", bufs=3)
            nc.vector.tensor_copy(out=eg16, in_=egB)
            gtail[bg] = {"egB": egB, "eg16": eg16}
            if DEBUG and bg == 0:
                nc.sync.dma_start(out=dbg["dbg_emax"][:], in_=emaxGs[bg])
                nc.sync.dma_start(out=dbg["dbg_gave"][:], in_=gaveGs[bg])
                nc.sync.dma_start(out=dbg["dbg_eg"][:], in_=egB)

        def g_tail_ra(bg):
            raB = sb_g.tile([128, 4], F32, tag="raB", name="raB", bufs=3)
            nc.vector.reciprocal(out=raB, in_=arawGs[bg][:, :, 0])
            gtail[bg]["raB"] = raB
            if DEBUG and bg == 0:
                nc.sync.dma_start(out=dbg["dbg_araw"][:], in_=arawGs[bg])

        def mol_tail_a(mol):
            """head-sum of gate numerators (PE) + reciprocal."""
            g0, g1 = gtail[2 * mol], gtail[2 * mol + 1]
            gd = ps_m.tile([32, 4], F32, tag="pm", name="gd")
            for r in range(4):
                nc.tensor.matmul(gd[:, r:r + 1], ssel_t,
                                 g0["eg16"][:, r:r + 1],
                                 start=True, stop=False)
                nc.tensor.matmul(gd[:, r:r + 1], ssel_t,
                                 g1["eg16"][:, r:r + 1],
                                 start=False, stop=True)
            rg = sb_g.tile([32, 4], F32, tag="rg", name="rg", bufs=2)
            nc.vector.reciprocal(out=rg, in_=gd)
            rg16 = sb_g.tile([32, 4], F16, tag="rg16", name="rg16", bufs=2)
            nc.vector.tensor_copy(out=rg16, in_=rg)
            return rg16

        def mol_tail_b(mol, rg16):
            """broadcast the head-sum back and scale the raw attention."""
            g0, g1 = gtail.pop(2 * mol), gtail.pop(2 * mol + 1)
            inv = ps_m.tile([128, 4], F32, tag="pm", name="inv")
            for r in range(4):
                nc.tensor.matmul(inv[:, r:r + 1], s2sel_t, rg16[:, r:r + 1],
                                 start=True, stop=True)
            invs = sb_g.tile([128, 4], F32, tag="invs", name="invs", bufs=2)
            nc.scalar.copy(out=invs, in_=inv)
            for gg, gt in ((0, g0), (1, g1)):
                bg = 2 * mol + gg
                t1 = sb_g.tile([128, 4], F32, tag="t1", name="t1", bufs=2)
                nc.gpsimd.tensor_mul(t1, invs, gt["raB"])
                c2 = sb_g.tile([128, 4], F32, tag="c2", name="c2", bufs=2)
                nc.gpsimd.tensor_mul(c2, t1, gt["egB"])
                outB = sb_g.tile([128, 4, 32], F32, tag="outB", name="outB",
                                 bufs=4)
                nc.gpsimd.tensor_mul(
                    outB, arawGs.pop(bg)[:, :, 1:33],
                    c2.unsqueeze(2).broadcast_to([128, 4, 32]))
                nc.sync.dma_start(out=o5[mol, gg], in_=outB)
                emaxGs.pop(bg, None)
                gaveGs.pop(bg, None)

        # ---------------- prologue ----------------
        # The cast ring (gpsimd) carries only x; all q/const traffic rides
        # the hardware ring.  The first two blocks' x loads are split per
        # row-phase and interleaved so transposes can start early.
        issue_qin(0)
        issue_qin(1)
        load_main_consts()
        xgt[0] = sb_x.tile([128, 4, 640], F16, tag="x16", name="x16")
        xgt[1] = sb_x.tile([128, 4, 640], F16, tag="x16", name="x16")
        for rr, bb in ((0, 0), (1, 0), (2, 0), (0, 1), (3, 0), (1, 1),
                       (2, 1), (3, 1)):
            mol_, G_ = divmod(bb, 2)
            nc.gpsimd.dma_start(out=xgt[bb][:, rr:rr + 1, :],
                                in_=xg[mol_, G_, :, rr:rr + 1, :])
        prepass(0, skip_qin=True)
        prepass(1, skip_qin=True)
        issue_q(0)
        nc.sync.dma_start(out=wv_t[:, 2], in_=wv_h[:, 2])
        nc.sync.dma_start(out=wv_t[:, 3], in_=wv_h[:, 3])
        for m in range(2, BM):
            issue_qin(m)
        for r in range(4):
            t_batch(0, r)

        NPH = 4 * NBLK
        pend_a = {}                     # stage idx -> mol (emit gd there)
        pend_b = {}                     # stage idx -> (mol, rg16)
        for i in range(NPH):
            bg, r = divmod(i, 4)
            mol, G = divmod(bg, 2)
            if r == 0 and bg + 2 < NBLK:
                issue_x(bg + 2)
            # PE: transposes for phase i+4's block, one batch per phase
            if i + 4 < NPH:
                bg2, r2 = divmod(i + 4, 4)
                t_batch(bg2, r2)
            if i > 0 and r == 0:
                g_tail_eg(bg - 1)      # independent of araw; feeds gd early
            if i > 0:
                phase_araw(i - 1)
                if r == 0:
                    g_tail_ra(bg - 1)
                    if G == 0 and mol > 0:
                        pend_a[i + 1] = mol - 1
            phase_matmuls(i)
            # gate PE matmuls ride at the end of later stages so their
            # vector/scalar inputs are always ready (no PE stall/p-state dip)
            if i in pend_a:
                m_ = pend_a.pop(i)
                pend_b[i + 1] = (m_, mol_tail_a(m_))
            if i in pend_b:
                m_, rg16_ = pend_b.pop(i)
                mol_tail_b(m_, rg16_)
            phase_front(i)
            phase_score(i)
            if i >= 4 and i % 2 == 0 and i // 2 < BM:
                prepass(i // 2, skip_qin=True)   # prepasses 2..7 interleaved
            if r == 1 and bg + 1 < NBLK:
                issue_q(bg + 1)
        g_tail_eg(NBLK - 1)            # before the last araw: shorter tail
        phase_araw(NPH - 1)
        g_tail_ra(NBLK - 1)
        mol_tail_b(BM - 1, mol_tail_a(BM - 1))

    nc.finalize()
    return nc


def _prep_consts(Wq, bq, Wk, bk, Wv, bv, Wam, bam, Wg, bg):
    Wq = np.asarray(Wq, np.float32)
    Wk = np.asarray(Wk, np.float32)
    Wv = np.asarray(Wv, np.float32)
    Wam = np.asarray(Wam, np.float32)
    wg = np.asarray(Wg, np.float32).reshape(-1)

    # q projection + gate-cur block-diagonal columns
    wq = np.zeros((128, 2, 264), np.float32)
    for c in range(2):
        wq[:, c, 0:256] = Wq[128 * c:128 * (c + 1), :]
        for p in range(128):
            ch = 128 * c + p
            wq[p, c, 256 + ch // 32] = wg[ch % 32]

    # K/M natural packs (row-chunked): pack[p, fc, c] = W[128*fc + p, c]
    wk_c = np.empty((128, 4, 256), np.float32)
    wm_c = np.empty((128, 4, 256), np.float32)
    for fc in range(4):
        rows = slice(128 * fc, 128 * (fc + 1))
        wk_c[:, fc, :] = Wk[rows, :]
        wm_c[:, fc, :] = Wam[rows, :]
    # V per-phase pack, each range's columns contiguous in (k, n) order
    wv_c = np.empty((128, 4, 4, 320), np.float32)
    for r in range(4):
        wa = 256 - 64 * r
        nA = wa // 32
        cols = np.empty((32, NEI), np.int64)
        for k in range(32):
            for n in range(NEI):
                sv = 32 * n + k
                cols[k, n] = 64 * r + sv if sv < wa else sv - wa
        order = np.concatenate([cols[:, 0:nA].reshape(-1),
                                cols[:, nA:NEI].reshape(-1)])
        for fc in range(4):
            rows = slice(128 * fc, 128 * (fc + 1))
            wv_c[:, r, fc, :] = Wv[rows, :][:, order]

    p = np.arange(128)
    ssel = (p[:, None] % 32 == np.arange(32)[None, :]).astype(np.float16)
    consts = {
        "ident": np.eye(128, dtype=np.float16),
        "wq": wq.astype(np.float16),
        "wk": wk_c.astype(np.float16),
        "wv": wv_c.astype(np.float16),
        "wm": wm_c.astype(np.float16),
        "wgav": (wg[64 + (p % 64)] / NEI).astype(np.float16).reshape(128, 1),
        "wge": np.tile(wg[32:64], (128, 1)).astype(np.float32),
        "ssel": ssel,
        "s2sel": ssel.T.copy(),
    }
    bg_val = float(np.asarray(bg).reshape(-1)[0])
    if bg_val != 0.0:
        consts["bgc"] = np.full((128, 1), bg_val, np.float32)
    with_bias = any(np.any(np.asarray(b) != 0) for b in (bq, bk, bv, bam))
    if with_bias:
        bkvm = np.stack([np.asarray(bv, np.float32),
                         np.asarray(bk, np.float32),
                         np.asarray(bam, np.float32)])[None]
        consts["bkvm"] = bkvm.astype(np.float16)
        bvp = np.empty((1, 4, 320), np.float32)
        bvf = np.asarray(bv, np.float32)
        for r in range(4):
            wa = 256 - 64 * r
            nA = wa // 32
            cols = np.empty((32, NEI), np.int64)
            for k in range(32):
                for n in range(NEI):
                    sv = 32 * n + k
                    cols[k, n] = 64 * r + sv if sv < wa else sv - wa
            order = np.concatenate([cols[:, 0:nA].reshape(-1),
                                    cols[:, nA:NEI].reshape(-1)])
            bvp[0, r, :] = bvf[order]
        consts["bvp"] = bvp.astype(np.float16)
        bqp = np.zeros((1, 264), np.float32)
        bqp[0, 0:256] = np.asarray(bq, np.float32)
        consts["bq"] = bqp.astype(np.float16)
        consts["ones"] = np.ones((1, 128), np.float16)
    return consts, with_bias, float(np.asarray(bg).reshape(-1)[0])


_CACHE = {}
TRACE = False       # set by test.py for profiling runs
LAST_RESULTS = None  # BassKernelResults from the most recent run


def kernel(input_multihead, input_q, Wq, bq, Wk, bk, Wv, bv, Wam, bam, Wg, bg):
    from concourse.bass_utils import run_bass_kernel_spmd

    consts, with_bias, bg_val = _prep_consts(
        Wq, bq, Wk, bk, Wv, bv, Wam, bam, Wg, bg)

    key = (with_bias, bg_val, DEBUG)
    if key not in _CACHE:
        _CACHE[key] = build_nc(with_bias, bg_val)
    nc = _CACHE[key]

    x = np.ascontiguousarray(np.asarray(input_multihead, np.float32))
    q = np.ascontiguousarray(np.asarray(input_q, np.float32))
    in_maps = []
    for c in range(N_CORES):
        m = {"x": x[BM * c:BM * (c + 1)], "qin": q[BM * c:BM * (c + 1)]}
        m.update(consts)
        in_maps.append(m)

    res = run_bass_kernel_spmd(nc, in_maps, list(range(N_CORES)), trace=TRACE)
    global LAST_RESULTS
    LAST_RESULTS = res
    return np.concatenate([res.results[c]["out"] for c in range(N_CORES)],
                          axis=0)
